# revision 1
# baseline (speedup 1.0000x reference)
"""Int8 AG-GEMM (x @ weight.T with per-row/per-col dequant + bias) on 8 TRN2
NeuronCores, computed with one level of Strassen (12.5% fewer MACs than the
plain bf16 GEMM, which already ran at 96% of the bf16 tensor roofline).

Sharding: 4 (M) x 2 (N) core grid. Core (h, s) owns output rows
[h*1024, (h+1)*1024) and cols [s*4096, (s+1)*4096). Within each core the
[1024, 8192] x [8192, 4096] product is computed as 7 Strassen products
Mi = A'_i @ B'_i with A'_i [512, 4096] and B'_i [4096, 2048]. The combo
operands (e.g. A11+A22) are sums/differences of two int8 values, i.e.
integers in [-256, 254] -- exactly representable in bf16 (integers up to
512 are exact), so the whole pipeline stays exact int arithmetic in the
PE's fp32 accumulator, like the plain kernel.

Per-core kernel layout (all operands host-prepped into SBUF tile layout):
  atl [7, 16, 128, 1024] bf16   : A'_i transposed, 16 chunks of 2 k-tiles
  btl [7, 16, 4, 128, 1024] bf16: B'_i tiles, n-tile-group major (16 groups
                                  of 128 n-cols), 4 quarters of 8 k-tiles
  isr [2, 128, 512] f32         : input_scale slabs per m-quadrant (broadcast)
  wsr, br [128, 32] f32         : weight_scale / bias, partition-major
Each core computes outT [4096, 1024] bf16 (transposed output shard):
  for each of 16 n-tile groups g and product i:
    psum[128 n, 512 m] = sum_kt B'tile[kt].T @ A'slab[kt]   (32 matmuls, FD 512)
  psums drain via DVE into 4 SBUF quadrant accumulators (Strassen combine);
  product order M1,M2,M3,M5,M4,M6,M7 completes quadrants incrementally so
  dequant (x isr, x ws + bias -> bf16) + output DMA spread across the run.
The host transposes each core's outT back and stitches the full [M, N].

The very first pass runs all 8 n-groups kt-outer in parallel (8 psum banks)
so the cold DMA queues see a uniform ~140GB/s demand instead of a 4MB burst
per group; steady-state passes run group-sequential, which drains each psum
while the next group's matmuls stream (no bank contention). A' chunks ride
the scalar HWDGE queue; B' quarters alternate between the sync HWDGE queue
and the gpsimd SWDGE queue by n-group parity; outputs ride the queue
opposite their group's B' queue. Pool depths are sized so every stream is
prefetched a full pass ahead (a shallower A pool measurably stalled the PE
at late pass boundaries).
"""

import numpy as np

M_FULL, K_FULL, N_FULL = 4096, 8192, 8192
N_CORES = 8
MSHARDS, NSHARDS = 4, 2
M_C = M_FULL // MSHARDS           # 1024 rows per core
N_C = N_FULL // NSHARDS           # 4096 cols per core
KH = K_FULL // 2                  # 4096, Strassen k-half
MQ = M_C // 2                     # 512, quadrant rows = matmul free dim
NQ = N_C // 2                     # 2048, quadrant cols
KT = KH // 128                    # 32 k-tiles per product
NG = N_C // 128                   # 32 n-tiles per core, 16 per quadrant
ACH = 16                          # A' chunks per product (2 kt each)
BCH = 4                           # B' slab quarters per n-group (8 kt each)

# Strassen: M1=(A11+A22)(B11+B22), M2=(A21+A22)B11, M3=A11(B12-B22),
# M4=A22(B21-B11), M5=(A11+A12)B22, M6=(A21-A11)(B11+B12), M7=(A12-A22)(B21+B22)
# C11=M1+M4-M5+M7, C12=M3+M5, C21=M2+M4, C22=M1-M2+M3+M6
FEEDS = {
    0: (("c11", 1), ("c22", 1)),
    1: (("c21", 1), ("c22", -1)),
    2: (("c12", 1), ("c22", 1)),
    3: (("c11", 1), ("c21", 1)),
    4: (("c11", -1), ("c12", 1)),
    5: (("c22", 1),),
    6: (("c11", 1),),
}
ORDER = [0, 1, 2, 4, 3, 5, 6]     # completes c12, c21, c22, c11 in turn
FLUSH = {4: "c12", 3: "c21", 5: "c22", 6: "c11"}
QPOS = {"c11": (0, 0), "c12": (0, 1), "c21": (1, 0), "c22": (1, 1)}


def build_nc():
    """Build the SPMD kernel graph (identical on all cores)."""
    import concourse.mybir as mybir
    import concourse.tile as tile
    from concourse import bacc

    bf16 = mybir.dt.bfloat16
    f32 = mybir.dt.float32

    nc = bacc.Bacc("TRN2", target_bir_lowering=False, debug=False,
                   num_devices=N_CORES)
    atl = nc.dram_tensor("atl", [7, ACH, 128, (KT // ACH) * MQ], bf16,
                         kind="ExternalInput")
    btl = nc.dram_tensor("btl", [7, NG // 2, BCH, 128, (KT // BCH) * 128],
                         bf16, kind="ExternalInput")
    isr = nc.dram_tensor("isr", [2, 128, MQ], f32, kind="ExternalInput")
    wsr = nc.dram_tensor("wsr", [128, NG], f32, kind="ExternalInput")
    br = nc.dram_tensor("br", [128, NG], f32, kind="ExternalInput")
    outt = nc.dram_tensor("outt", [N_C, M_C], bf16, kind="ExternalOutput")

    kpc = KT // ACH               # k-tiles per A' chunk
    kpb = KT // BCH               # k-tiles per B' quarter

    with tile.TileContext(nc) as tc:
        with (
            tc.tile_pool(name="const", bufs=1) as cpool,
            tc.tile_pool(name="astream", bufs=32) as apool,
            tc.tile_pool(name="bstream", bufs=34) as bpool,
            tc.tile_pool(name="accum", bufs=32) as accpool,
            tc.tile_pool(name="psum", bufs=8, space="PSUM") as ppool,
            tc.tile_pool(name="t1", bufs=2) as t1pool,
            tc.tile_pool(name="osb", bufs=2) as opool,
        ):
            isr_sb = [cpool.tile([128, MQ], f32, name=f"isr{q}")
                      for q in range(2)]
            ws_sb = cpool.tile([128, NG], f32)
            bia_sb = cpool.tile([128, NG], f32)

            def drain_and_flush(i, nthalf, nt, ps, accs):
                g = nthalf * 8 + nt
                oeng = nc.gpsimd if nt % 2 == 0 else nc.sync
                for q, sign in FEEDS[i]:
                    key = (q, nt)
                    if key not in accs:
                        acc = accpool.tile([128, MQ], f32, tag="acc")
                        accs[key] = acc
                        nc.vector.tensor_copy(acc[:], ps[:])
                    else:
                        nc.vector.tensor_tensor(
                            accs[key][:], accs[key][:], ps[:],
                            mybir.AluOpType.add if sign > 0
                            else mybir.AluOpType.subtract)
                if i in FLUSH:
                    qname = FLUSH[i]
                    qm, qn = QPOS[qname]
                    acc = accs[(qname, nt)]
                    t1 = t1pool.tile([128, MQ], f32)
                    nc.vector.tensor_tensor(
                        t1[:], acc[:], isr_sb[qm][:], mybir.AluOpType.mult)
                    j = qn * (NG // 2) + g
                    ob = opool.tile([128, MQ], bf16)
                    nc.vector.tensor_scalar(
                        ob[:], t1[:],
                        ws_sb[:, j:j + 1], bia_sb[:, j:j + 1],
                        mybir.AluOpType.mult, mybir.AluOpType.add)
                    row = qn * NQ + g * 128
                    oeng.dma_start(
                        outt.ap()[row:row + 128, qm * MQ:(qm + 1) * MQ],
                        ob[:])

            def mm(ps, b_sb, a_sb, kt):
                nc.tensor.matmul(
                    ps[:],
                    b_sb[kt // kpb][:, (kt % kpb) * 128:(kt % kpb + 1) * 128],
                    a_sb[kt // kpc][:, (kt % kpc) * MQ:(kt % kpc + 1) * MQ],
                    start=(kt == 0),
                    stop=(kt == KT - 1),
                )

            def b_tiles(i, g, q):
                b = bpool.tile([128, kpb * 128], bf16, tag="bsl")
                beng = nc.sync if g % 2 == 0 else nc.gpsimd
                beng.dma_start(b[:], btl.ap()[i, g, q])
                return b

            # --- first pass: all 8 groups run kt-outer in parallel so the
            # cold DMA queues see a uniform ~140GB/s demand instead of a
            # 4MB burst for each group's full-K sweep. B quarters are
            # emitted quarter-major to match the kt-major consumption.
            # PE clock warm-up: ~140 dependency-free dummy matmuls starting
            # right after the preamble keep the PE busy through the HAM
            # activity window, so it reaches 2.4GHz (K=8/8) before the first
            # real matmul's data lands (~11us). Without this the first ~13us
            # of real matmuls run at the cold 1.2GHz clock (~4us lost).
            junk = cpool.tile([128, 128], bf16)
            nc.vector.memset(junk[:], 0.0)
            scratch = ppool.tile([128, MQ], f32, name="ps")
            for w in range(140):
                nc.tensor.matmul(scratch[:, 0:128], junk[:], junk[:],
                                 start=True, stop=True)

            i0 = ORDER[0]
            accs0 = {}
            a_sb = []
            b0 = {}
            for c in range(ACH):
                a = apool.tile([128, kpc * MQ], bf16, tag="ach")
                nc.scalar.dma_start(a[:], atl.ap()[i0, c])
                a_sb.append(a)
            for q in range(2):
                nc.scalar.dma_start(isr_sb[q][:], isr.ap()[q])
            nc.scalar.dma_start(ws_sb[:], wsr.ap())
            nc.scalar.dma_start(bia_sb[:], br.ap())
            for q in range(BCH):
                for g in range(8):
                    b0[(g, q)] = b_tiles(i0, g, q)
            ps0 = [ppool.tile([128, MQ], f32, name="ps")
                   for j in range(8)]
            for kt in range(KT):
                for nt in range(8):
                    mm(ps0[nt], [b0[(nt, q)] for q in range(BCH)], a_sb, kt)
            for nt in range(8):
                drain_and_flush(i0, 0, nt, ps0[nt], accs0)

            # --- steady state: group-sequential (better drain pipelining)
            for nthalf in range(2):
                accs = accs0 if nthalf == 0 else {}
                for i in ORDER:
                    if nthalf == 0 and i == i0:
                        continue
                    a_sb = []
                    for c in range(ACH):
                        a = apool.tile([128, kpc * MQ], bf16, tag="ach")
                        nc.scalar.dma_start(a[:], atl.ap()[i, c])
                        a_sb.append(a)
                    for nt in range(8):
                        g = nthalf * 8 + nt
                        b_sb = [b_tiles(i, g, q) for q in range(BCH)]
                        ps = ppool.tile([128, MQ], f32)
                        for kt in range(KT):
                            mm(ps, b_sb, a_sb, kt)
                        drain_and_flush(i, nthalf, nt, ps, accs)

    nc.compile()
    return nc


def prep_in_maps(x, weight, bias, input_scale, weight_scale):
    """Host-side Strassen operand prep. Returns in_maps (len 8)."""
    import ml_dtypes

    bf16 = ml_dtypes.bfloat16
    xT = np.ascontiguousarray(x.T).astype(np.int16)          # [K, M]
    wT = np.ascontiguousarray(weight.astype(np.int16))       # [N, K]; B = wT.T

    def a_layout(a):
        # A'_i.T [KH, MQ] int16 -> [ACH, 128, kpc*MQ] bf16
        kpc = KT // ACH
        return np.ascontiguousarray(
            a.reshape(ACH, kpc, 128, MQ).transpose(0, 2, 1, 3)
            .reshape(ACH, 128, kpc * MQ)).astype(bf16)

    def b_layout(b):
        # B'_i [KH, NQ] int16 -> [16, BCH, 128, kpb*128] bf16
        kpb = KT // BCH
        return np.ascontiguousarray(
            b.reshape(BCH, kpb, 128, NG // 2, 128).transpose(3, 0, 2, 1, 4)
            .reshape(NG // 2, BCH, 128, kpb * 128)).astype(bf16)

    atls = []
    for h in range(MSHARDS):
        mb = h * M_C
        a11 = xT[0:KH, mb:mb + MQ]
        a12 = xT[KH:K_FULL, mb:mb + MQ]
        a21 = xT[0:KH, mb + MQ:mb + M_C]
        a22 = xT[KH:K_FULL, mb + MQ:mb + M_C]
        combos = [a11 + a22, a21 + a22, a11, a22, a11 + a12,
                  a21 - a11, a12 - a22]
        atls.append(np.stack([a_layout(a) for a in combos]))

    btls = []
    for s in range(NSHARDS):
        nb = s * N_C
        b11 = np.ascontiguousarray(wT[nb:nb + NQ, 0:KH].T)
        b12 = np.ascontiguousarray(wT[nb + NQ:nb + N_C, 0:KH].T)
        b21 = np.ascontiguousarray(wT[nb:nb + NQ, KH:K_FULL].T)
        b22 = np.ascontiguousarray(wT[nb + NQ:nb + N_C, KH:K_FULL].T)
        combos = [b11 + b22, b11, b12 - b22, b21 - b11, b22,
                  b11 + b12, b21 + b22]
        btls.append(np.stack([b_layout(b) for b in combos]))

    in_maps = []
    for c in range(N_CORES):
        h, s = c // NSHARDS, c % NSHARDS
        mb, nb = h * M_C, s * N_C
        isr = np.ascontiguousarray(np.broadcast_to(
            input_scale[mb:mb + M_C].astype(np.float32).reshape(2, 1, MQ),
            (2, 128, MQ)))
        wsr = np.ascontiguousarray(
            weight_scale[nb:nb + N_C].astype(np.float32).reshape(NG, 128).T)
        brr = np.ascontiguousarray(
            bias[nb:nb + N_C].astype(np.float32).reshape(NG, 128).T)
        in_maps.append({
            "atl": atls[h],
            "btl": btls[s],
            "isr": isr,
            "wsr": wsr,
            "br": brr,
        })
    return in_maps


def run(x, weight, bias, input_scale, weight_scale, trace=False):
    """Run the SPMD kernel; returns (out [M, N] bf16, BassKernelResults)."""
    from concourse.bass_utils import run_bass_kernel_spmd

    in_maps = prep_in_maps(x, weight, bias, input_scale, weight_scale)
    nc = build_nc()
    res = run_bass_kernel_spmd(nc, in_maps, list(range(N_CORES)), trace=trace)

    import ml_dtypes
    out = np.empty((M_FULL, N_FULL), dtype=ml_dtypes.bfloat16)
    for c in range(N_CORES):
        h, s = c // NSHARDS, c % NSHARDS
        mb, nb = h * M_C, s * N_C
        out[mb:mb + M_C, nb:nb + N_C] = res.results[c]["outt"].T
    return out, res


def kernel(x, weight, bias, input_scale, weight_scale):
    x, weight, bias, input_scale, weight_scale = (
        np.asarray(a) for a in (x, weight, bias, input_scale, weight_scale))
    out, _ = run(x, weight, bias, input_scale, weight_scale, trace=False)
    return out

